# revision 2
# baseline (speedup 1.0000x reference)
"""Trainium2 Bass kernel for nn_DecoderGRUWeighted (batch-1 GRU decoder step).

Strategy (8 NeuronCores, SPMD):
  - Vocab dim of the output projection Wo (50257x1024, the dominant memory
    traffic) is sharded 8 ways; each core computes a [1, V/8] logits slice.
  - The small attention/GRU chain is sharded too: Ww/encoder by L, Wc by
    output rows, Wih/Whh by contraction dim; partial results are combined
    with tiny collectives (2 AllGathers + 2 AllReduces).
  - All length-D vectors live on chip in "column layout": SBUF tile
    [128, D/128] with element d at (partition d%128, column d//128), so every
    matvec uses natural [128,128] weight tiles as the PE stationary operand
    and [128,1] vector columns as the moving operand, with no transposes.
  - log_softmax is computed without max subtraction (logits are O(1) here):
    out = logits - ln(sum_exp), with the global sum reduced via AllGather.
  - Big weights (Wo^T, Wih^T, Whh^T) are stored bf16 on chip to halve HBM
    traffic; everything else stays f32.
"""

import sys

if "/opt/trn_rl_repo" not in sys.path:
    sys.path.insert(0, "/opt/trn_rl_repo")

import numpy as np
import ml_dtypes

H = 1024
V = 50257
L = 512
NCORES = 8
V_SH = 6283          # ceil(V / 8); global pad = 50264 (7 zero rows on core 7)
V_PAD = V_SH * NCORES
CH = 512             # logits chunk (one PSUM bank of f32)
N_CH = 13            # 12*512 + 139
SUB_SPLIT = 7 * CH   # final log-softmax subtract: DVE does [0:3584), ACT the rest

_BF16 = ml_dtypes.bfloat16

_cache: dict = {}


def _build():
    import concourse.bacc as bacc
    import concourse.tile as tile
    from concourse import mybir

    f32 = mybir.dt.float32
    bf16 = mybir.dt.bfloat16
    AF = mybir.ActivationFunctionType

    nc = bacc.Bacc("TRN2", target_bir_lowering=False, debug=False,
                   num_devices=NCORES)

    # ---- I/O ------------------------------------------------------------
    # vecs columns: 0:16 cat1 | 16:24 embed | 24:32 hidden | 32 ones |
    #               33:49 bih+bhh (r,z) | 49:57 bih_n | 57:65 bhh_n |
    #               65 bw shard (partitions 0:64) | 66 bc shard
    vecs = nc.dram_tensor("vecs", [128, 68], f32, kind="ExternalInput")
    aux_row = nc.dram_tensor("aux_row", [1, 128], f32, kind="ExternalInput")
    wwt = nc.dram_tensor("wwt", [2048, 64], f32, kind="ExternalInput")
    enc = nc.dram_tensor("enc", [64, 1024], f32, kind="ExternalInput")
    wct = nc.dram_tensor("wct", [2048, 128], f32, kind="ExternalInput")
    wiht = nc.dram_tensor("wiht", [128, 3072], f32, kind="ExternalInput")
    whht = nc.dram_tensor("whht", [128, 3072], f32, kind="ExternalInput")
    hloc = nc.dram_tensor("hloc", [128, 1], f32, kind="ExternalInput")
    wot = nc.dram_tensor("wot", [1024, V_SH], bf16, kind="ExternalInput")
    bo_sh = nc.dram_tensor("bo_sh", [1, V_SH], f32, kind="ExternalInput")

    out_sl = nc.dram_tensor("out_sl", [1, V_SH], f32, kind="ExternalOutput")
    hnew_o = nc.dram_tensor("hnew_o", [128, 8], f32, kind="ExternalOutput")
    w_o = nc.dram_tensor("w_o", [1, 512], f32, kind="ExternalOutput")

    RG = [list(range(NCORES))]

    with tile.TileContext(nc) as tc:
        with (
            tc.tile_pool(name="const", bufs=1) as cpool,
            tc.tile_pool(name="work", bufs=2) as wpool,
            tc.tile_pool(name="lgp", bufs=6, space="PSUM") as lg_pool,
            tc.tile_pool(name="spp", bufs=2, space="PSUM") as sp_pool,
            tc.tile_pool(name="dram", bufs=1, space="DRAM") as dpool,
        ):
            # ---- stage the small inputs (chain-critical DMAs first) -----
            vecs_sb = cpool.tile([128, 68], f32)
            nc.sync.dma_start(vecs_sb[:], vecs.ap())
            aux_sb = cpool.tile([1, 128], f32)
            nc.sync.dma_start(aux_sb[:], aux_row.ap())
            wwt_sb = cpool.tile([128, 16, 64], f32)
            nc.sync.dma_start(
                wwt_sb[:], wwt.ap().rearrange("(c p) f -> p c f", p=128))
            enc_sb = cpool.tile([64, 1024], f32)
            nc.sync.dma_start(enc_sb[:], enc.ap())
            wct_sb = cpool.tile([128, 16, 128], f32)
            nc.sync.dma_start(
                wct_sb[:], wct.ap().rearrange("(c p) f -> p c f", p=128))
            wiht_sb = cpool.tile([128, 3072], f32)
            nc.sync.dma_start(wiht_sb[:], wiht.ap())
            whht_sb = cpool.tile([128, 3072], f32)
            nc.sync.dma_start(whht_sb[:], whht.ap())
            hloc_sb = cpool.tile([128, 1], f32)
            nc.sync.dma_start(hloc_sb[:], hloc.ap())
            bo_sb = cpool.tile([1, V_SH], f32)
            nc.sync.dma_start(bo_sb[:], bo_sh.ap())

            # ---- big Wo^T shard: 8 contraction tiles, streamed early ----
            wot_sb = cpool.tile([128, 8, V_SH], bf16)
            for k in range(8):
                nc.sync.dma_start(
                    wot_sb[:, k, :], wot.ap()[k * 128:(k + 1) * 128, :])

            ones_col = vecs_sb[:, 32:33]

            # ---- attention scores s = Ww_sh @ cat1 + bw_sh  ([64,1]) ----
            s_ps = sp_pool.tile([64, 1], f32, tag="sp")
            for c in range(16):
                nc.tensor.matmul(s_ps[:], wwt_sb[:, c, :],
                                 vecs_sb[:, c:c + 1],
                                 start=(c == 0), stop=(c == 15))
            s_sb = wpool.tile([64, 1], f32, tag="s_sb")
            nc.scalar.activation(s_sb[:], s_ps[:], AF.Identity,
                                 bias=vecs_sb[0:64, 65:66])
            exp_loc = wpool.tile([64, 1], f32, tag="exp_loc")
            nc.scalar.activation(exp_loc[:], s_sb[:], AF.Exp)
            # local sum of exp over the 64 partitions (matmul with ones)
            S_ps = sp_pool.tile([1, 1], f32, tag="sp")
            nc.tensor.matmul(S_ps[:], ones_col[0:64, :], exp_loc[:],
                             start=True, stop=True)
            spack = wpool.tile([1, 8], f32, tag="spack")
            nc.vector.memset(spack[:], 0.0)
            nc.scalar.copy(spack[0:1, 0:1], S_ps[:])

            # ---- AllGather #1: [s_local(64) | S_local | pad] -> [8,72] --
            cc1_in = dpool.tile([1, 72], f32)
            cc1_out = dpool.tile([8, 72], f32)
            nc.sync.dma_start(cc1_in[0:1, 0:64], s_sb[:])
            nc.sync.dma_start(cc1_in[0:1, 64:72], spack[:])
            nc.gpsimd.collective_compute(
                "AllGather", mybir.AluOpType.bypass, replica_groups=RG,
                ins=[cc1_in.opt()], outs=[cc1_out.opt()])

            # global softmax denominator
            srow8 = wpool.tile([1, 8], f32, tag="srow8")
            nc.sync.dma_start(srow8[:], cc1_out.opt()[:, 64:65])
            S_tot = wpool.tile([1, 1], f32, tag="S_tot")
            nc.vector.reduce_sum(S_tot[:], srow8[:], axis=mybir.AxisListType.X)
            rinv = wpool.tile([1, 1], f32, tag="rinv")
            nc.vector.reciprocal(rinv[:], S_tot[:])
            rinv64_ps = sp_pool.tile([64, 1], f32, tag="sp")
            nc.tensor.matmul(rinv64_ps[:], aux_sb[0:1, 0:64], rinv[:],
                             start=True, stop=True)
            rinv64 = wpool.tile([64, 1], f32, tag="rinv64")
            nc.scalar.copy(rinv64[:], rinv64_ps[:])
            w_loc = wpool.tile([64, 1], f32, tag="w_loc")
            nc.vector.tensor_scalar_mul(w_loc[:], exp_loc[:], rinv64[:])

            # ---- weighted context (partial over local L rows) -----------
            wctx_ps = sp_pool.tile([128, 8], f32, tag="sp")
            for m in range(8):
                nc.tensor.matmul(wctx_ps[:, m:m + 1],
                                 enc_sb[:, m * 128:(m + 1) * 128], w_loc[:],
                                 start=True, stop=True)
            wctx_sb = wpool.tile([128, 8], f32, tag="wctx_sb")
            nc.scalar.copy(wctx_sb[:], wctx_ps[:])

            # ---- AllReduce #1: wctx ------------------------------------
            ccw_in = dpool.tile([128, 8], f32)
            ccw_out = dpool.tile([128, 8], f32)
            nc.sync.dma_start(ccw_in[:], wctx_sb[:])
            nc.gpsimd.collective_compute(
                "AllReduce", mybir.AluOpType.add, replica_groups=RG,
                ins=[ccw_in.opt()], outs=[ccw_out.opt()])
            wctx_col = wpool.tile([128, 8], f32, tag="wctx_col")
            nc.sync.dma_start(wctx_col[:], ccw_out.opt())

            # ---- out = relu(Wc_sh @ [embed; wctx] + bc_sh)  ([128,1]) ---
            o_ps = sp_pool.tile([128, 1], f32, tag="sp")
            for c in range(16):
                rhs = vecs_sb[:, 16 + c:17 + c] if c < 8 else \
                    wctx_col[:, c - 8:c - 7]
                nc.tensor.matmul(o_ps[:], wct_sb[:, c, :], rhs,
                                 start=(c == 0), stop=(c == 15))
            relu_bf = wpool.tile([128, 1], f32, tag="relu_bf")
            nc.scalar.activation(relu_bf[:], o_ps[:], AF.Relu,
                                 bias=vecs_sb[:, 66:67])

            # ---- GRU gate partials (contraction over local H slice) -----
            gig_ps = sp_pool.tile([128, 24], f32, tag="sp")
            for m in range(24):
                nc.tensor.matmul(gig_ps[:, m:m + 1],
                                 wiht_sb[:, m * 128:(m + 1) * 128],
                                 relu_bf[:], start=True, stop=True)
            ghh_ps = sp_pool.tile([128, 24], f32, tag="sp")
            for m in range(24):
                nc.tensor.matmul(ghh_ps[:, m:m + 1],
                                 whht_sb[:, m * 128:(m + 1) * 128],
                                 hloc_sb[:], start=True, stop=True)
            gigh_sb = wpool.tile([128, 48], f32, tag="gigh_sb")
            nc.scalar.copy(gigh_sb[:, 0:24], gig_ps[:])
            nc.vector.tensor_copy(gigh_sb[:, 24:48], ghh_ps[:])

            # ---- AllReduce #2: [gi | gh] partials ----------------------
            ccg_in = dpool.tile([128, 48], f32)
            ccg_out = dpool.tile([128, 48], f32)
            nc.sync.dma_start(ccg_in[:], gigh_sb[:])
            nc.gpsimd.collective_compute(
                "AllReduce", mybir.AluOpType.add, replica_groups=RG,
                ins=[ccg_in.opt()], outs=[ccg_out.opt()])
            gigh = wpool.tile([128, 48], f32, tag="gigh")
            nc.sync.dma_start(gigh[:], ccg_out.opt())

            # ---- gates: r,z = sig(gi+gh+b); n = tanh(gi_n+b + r*(gh_n+b))
            trz = wpool.tile([128, 16], f32, tag="trz")
            nc.vector.tensor_add(trz[:], gigh[:, 0:16], gigh[:, 24:40])
            trz2 = wpool.tile([128, 16], f32, tag="trz2")
            nc.vector.tensor_add(trz2[:], trz[:], vecs_sb[:, 33:49])
            rz = wpool.tile([128, 16], f32, tag="rz")
            nc.scalar.activation(rz[:], trz2[:], AF.Sigmoid)
            ghn = wpool.tile([128, 8], f32, tag="ghn")
            nc.vector.tensor_add(ghn[:], gigh[:, 40:48], vecs_sb[:, 57:65])
            tn = wpool.tile([128, 8], f32, tag="tn")
            nc.vector.tensor_mul(tn[:], rz[:, 0:8], ghn[:])
            tn2 = wpool.tile([128, 8], f32, tag="tn2")
            nc.vector.tensor_add(tn2[:], tn[:], gigh[:, 16:24])
            tn3 = wpool.tile([128, 8], f32, tag="tn3")
            nc.vector.tensor_add(tn3[:], tn2[:], vecs_sb[:, 49:57])
            nn_t = wpool.tile([128, 8], f32, tag="nn_t")
            nc.scalar.activation(nn_t[:], tn3[:], AF.Tanh)
            dd = wpool.tile([128, 8], f32, tag="dd")
            nc.vector.tensor_sub(dd[:], vecs_sb[:, 24:32], nn_t[:])
            ee = wpool.tile([128, 8], f32, tag="ee")
            nc.vector.tensor_mul(ee[:], rz[:, 8:16], dd[:])
            hnew = wpool.tile([128, 8], f32, tag="hnew")
            nc.vector.tensor_add(hnew[:], nn_t[:], ee[:])
            nc.sync.dma_start(hnew_o.ap(), hnew[:])
            hnew_bf = wpool.tile([128, 8], bf16, tag="hnew_bf")
            nc.vector.tensor_copy(hnew_bf[:], hnew[:])

            # ---- logits slice: 13 chunks of <=512, accumulate 8 k-tiles -
            logits_row = cpool.tile([1, V_SH], f32)
            sumexps = cpool.tile([1, 16], f32)
            for c in range(N_CH):
                c0 = c * CH
                csz = min(CH, V_SH - c0)
                lg_ps = lg_pool.tile([1, CH], f32, tag="lg")
                for k in range(8):
                    nc.tensor.matmul(lg_ps[0:1, 0:csz],
                                     hnew_bf[:, k:k + 1],
                                     wot_sb[:, k, c0:c0 + csz],
                                     start=(k == 0), stop=(k == 7))
                # logits = psum + bo  (into the row buffer), then exp+accum
                nc.vector.tensor_add(logits_row[0:1, c0:c0 + csz],
                                     lg_ps[0:1, 0:csz],
                                     bo_sb[0:1, c0:c0 + csz])
                etmp = wpool.tile([1, CH], f32, tag="etmp")
                nc.scalar.activation(etmp[0:1, 0:csz],
                                     logits_row[0:1, c0:c0 + csz], AF.Exp,
                                     accum_out=sumexps[0:1, c:c + 1])

            sumloc = wpool.tile([1, 1], f32, tag="sumloc")
            nc.vector.reduce_sum(sumloc[:], sumexps[0:1, 0:N_CH],
                                 axis=mybir.AxisListType.X)
            spack2 = wpool.tile([1, 8], f32, tag="spack2")
            nc.vector.memset(spack2[:], 0.0)
            nc.vector.tensor_copy(spack2[0:1, 0:1], sumloc[:])

            # ---- AllGather #2: logit sumexp ----------------------------
            ccs_in = dpool.tile([1, 8], f32)
            ccs_out = dpool.tile([8, 8], f32)
            nc.sync.dma_start(ccs_in[:], spack2[:])
            nc.gpsimd.collective_compute(
                "AllGather", mybir.AluOpType.bypass, replica_groups=RG,
                ins=[ccs_in.opt()], outs=[ccs_out.opt()])
            s2row = wpool.tile([1, 8], f32, tag="s2row")
            nc.sync.dma_start(s2row[:], ccs_out.opt()[:, 0:1])
            S2 = wpool.tile([1, 1], f32, tag="S2")
            nc.vector.reduce_sum(S2[:], s2row[:], axis=mybir.AxisListType.X)
            logS = wpool.tile([1, 1], f32, tag="logS")
            nc.scalar.activation(logS[:], S2[:], AF.Ln)
            nlogS = wpool.tile([1, 1], f32, tag="nlogS")
            nc.scalar.activation(nlogS[:], logS[:], AF.Copy, scale=-1.0)

            # out = logits - log(sum); split across DVE and ACT
            nc.vector.tensor_scalar_sub(logits_row[0:1, 0:SUB_SPLIT],
                                        logits_row[0:1, 0:SUB_SPLIT],
                                        logS[:])
            nc.scalar.activation(logits_row[0:1, SUB_SPLIT:V_SH],
                                 logits_row[0:1, SUB_SPLIT:V_SH],
                                 AF.Identity, bias=nlogS[:])
            nc.sync.dma_start(out_sl.ap(), logits_row[:])

            # ---- attention weights output (off the critical path) -------
            srow_all = wpool.tile([1, 576], f32, tag="srow_all")
            nc.sync.dma_start(srow_all[:], cc1_out.opt())
            eall = wpool.tile([1, 512], f32, tag="eall")
            nc.scalar.activation(
                eall[0:1, :].rearrange("p (r m) -> p r m", m=64),
                srow_all[0:1, :].rearrange("p (r m) -> p r m", m=72)[:, :, 0:64],
                AF.Exp)
            wrow = wpool.tile([1, 512], f32, tag="wrow")
            nc.vector.tensor_scalar_mul(wrow[:], eall[:], rinv[:])
            nc.sync.dma_start(w_o.ap(), wrow[:])

    nc.compile()
    return nc


def _col(v, ncols):
    return np.ascontiguousarray(v.reshape(ncols, 128).T)


def _prep_in_maps(inputs):
    f32 = np.float32
    x = np.asarray(inputs["x"]).reshape(-1)
    hidden = np.asarray(inputs["hidden"], f32).reshape(H)
    enc_full = np.asarray(inputs["encoder_outputs"], f32)
    emb = np.asarray(inputs["emb"], f32)
    Ww = np.asarray(inputs["Ww"], f32)
    bw = np.asarray(inputs["bw"], f32)
    Wc = np.asarray(inputs["Wc"], f32)
    bc = np.asarray(inputs["bc"], f32)
    Wih = np.asarray(inputs["Wih"], f32)
    Whh = np.asarray(inputs["Whh"], f32)
    bih = np.asarray(inputs["bih"], f32)
    bhh = np.asarray(inputs["bhh"], f32)
    Wo = np.asarray(inputs["Wo"], f32)
    bo = np.asarray(inputs["bo"], f32)

    embed = emb[int(x[0])]
    cat1 = np.concatenate([embed, hidden])

    vecs = np.zeros((128, 68), f32)
    vecs[:, 0:16] = _col(cat1, 16)
    vecs[:, 16:24] = _col(embed, 8)
    vecs[:, 24:32] = _col(hidden, 8)
    vecs[:, 32] = 1.0
    vecs[:, 33:49] = _col((bih + bhh)[0:2048], 16)
    vecs[:, 49:57] = _col(bih[2048:], 8)
    vecs[:, 57:65] = _col(bhh[2048:], 8)

    aux = np.ones((1, 128), f32)

    pad = V_PAD - V
    Wo_pad = np.concatenate([Wo, np.zeros((pad, H), f32)], axis=0)
    bo_pad = np.concatenate([bo, np.full((pad,), -1e4, f32)])

    in_maps = []
    for r in range(NCORES):
        vr = vecs.copy()
        vr[0:64, 65] = bw[r * 64:(r + 1) * 64]
        vr[:, 66] = bc[r * 128:(r + 1) * 128]
        hs = slice(r * 128, (r + 1) * 128)
        vs = slice(r * V_SH, (r + 1) * V_SH)
        in_maps.append({
            "vecs": vr,
            "aux_row": aux,
            "wwt": np.ascontiguousarray(Ww[r * 64:(r + 1) * 64, :].T),
            "enc": np.ascontiguousarray(enc_full[r * 64:(r + 1) * 64, :]),
            "wct": np.ascontiguousarray(Wc[hs, :].T),
            "wiht": np.ascontiguousarray(Wih[:, hs].T),
            "whht": np.ascontiguousarray(Whh[:, hs].T),
            "hloc": hidden[hs].reshape(128, 1).copy(),
            "wot": np.ascontiguousarray(Wo_pad[vs, :].T).astype(_BF16),
            "bo_sh": bo_pad[vs].reshape(1, V_SH),
        })
    return in_maps


def _get_nc():
    if "nc" not in _cache:
        _cache["nc"] = _build()
    return _cache["nc"]


def _assemble(results):
    out = np.concatenate(
        [results[r]["out_sl"].reshape(-1) for r in range(NCORES)])[:V]
    out = np.ascontiguousarray(out.reshape(1, V), dtype=np.float32)
    h_new = np.ascontiguousarray(
        results[0]["hnew_o"].T.reshape(1, 1, H), dtype=np.float32)
    weights = np.ascontiguousarray(
        results[0]["w_o"].reshape(1, L), dtype=np.float32)
    return out, h_new, weights


def kernel(**inputs):
    from concourse.bass_utils import run_bass_kernel_spmd

    nc = _get_nc()
    in_maps = _prep_in_maps(inputs)
    res = run_bass_kernel_spmd(nc, in_maps, list(range(NCORES)))
    return _assemble(res.results)


# revision 3
# speedup vs baseline: 1.4116x; 1.4116x over previous
"""Trainium2 Bass kernel for nn_DecoderGRUWeighted (batch-1 GRU decoder step).

Strategy (8 NeuronCores, SPMD):
  - Vocab dim of the output projection Wo (50257x1024, the dominant memory
    traffic) is sharded 8 ways; each core computes a [1, V/8] logits slice.
  - The small attention/GRU chain is sharded too: Ww/encoder by L, Wc by
    output rows, Wih/Whh by contraction dim; partial results are combined
    with tiny collectives (2 AllGathers + 2 AllReduces).
  - All length-D vectors live on chip in "column layout": SBUF tile
    [128, D/128] with element d at (partition d%128, column d//128), so every
    matvec uses natural [128,128] weight tiles as the PE stationary operand
    and [128,1] vector columns as the moving operand, with no transposes.
  - log_softmax is computed without max subtraction (logits are O(1) here):
    out = logits - ln(sum_exp), with the global sum reduced via AllGather.
  - Big weights (Wo^T, Wih^T, Whh^T) are stored bf16 on chip to halve HBM
    traffic; everything else stays f32.
"""

import sys

if "/opt/trn_rl_repo" not in sys.path:
    sys.path.insert(0, "/opt/trn_rl_repo")

import numpy as np
import ml_dtypes

H = 1024
V = 50257
L = 512
NCORES = 8
V_SH = 6283          # ceil(V / 8); global pad = 50264 (7 zero rows on core 7)
V_PAD = V_SH * NCORES
CH = 512             # logits chunk (one PSUM bank of f32)
N_CH = 13            # 12*512 + 139
SUB_SPLIT = 7 * CH   # final log-softmax subtract: DVE does [0:3584), ACT the rest

_BF16 = ml_dtypes.bfloat16

_cache: dict = {}


def _build():
    import concourse.bacc as bacc
    import concourse.tile as tile
    from concourse import mybir

    f32 = mybir.dt.float32
    bf16 = mybir.dt.bfloat16
    AF = mybir.ActivationFunctionType

    nc = bacc.Bacc("TRN2", target_bir_lowering=False, debug=False,
                   num_devices=NCORES)

    # ---- I/O ------------------------------------------------------------
    # vecs columns: 0:16 cat1 | 16:24 embed | 24:32 hidden | 32 ones |
    #               33:49 bih+bhh (r,z) | 49:57 bih_n | 57:65 bhh_n |
    #               65 bw shard (partitions 0:64) | 66 bc shard
    vecs = nc.dram_tensor("vecs", [128, 68], f32, kind="ExternalInput")
    aux_row = nc.dram_tensor("aux_row", [1, 128], f32, kind="ExternalInput")
    wwt = nc.dram_tensor("wwt", [2048, 64], f32, kind="ExternalInput")
    enc = nc.dram_tensor("enc", [64, 1024], f32, kind="ExternalInput")
    wct = nc.dram_tensor("wct", [2048, 128], f32, kind="ExternalInput")
    wiht = nc.dram_tensor("wiht", [128, 3072], f32, kind="ExternalInput")
    whht = nc.dram_tensor("whht", [128, 3072], f32, kind="ExternalInput")
    hloc = nc.dram_tensor("hloc", [128, 1], f32, kind="ExternalInput")
    wot = nc.dram_tensor("wot", [1024, V_SH], bf16, kind="ExternalInput")
    bo_sh = nc.dram_tensor("bo_sh", [1, V_SH], f32, kind="ExternalInput")

    out_sl = nc.dram_tensor("out_sl", [1, V_SH], f32, kind="ExternalOutput")
    hnew_o = nc.dram_tensor("hnew_o", [128, 8], f32, kind="ExternalOutput")
    w_o = nc.dram_tensor("w_o", [1, 512], f32, kind="ExternalOutput")

    RG = [list(range(NCORES))]

    with tile.TileContext(nc) as tc:
        with (
            tc.tile_pool(name="const", bufs=1) as cpool,
            tc.tile_pool(name="work", bufs=2) as wpool,
            tc.tile_pool(name="lgp", bufs=6, space="PSUM") as lg_pool,
            tc.tile_pool(name="spp", bufs=2, space="PSUM") as sp_pool,
            tc.tile_pool(name="dram", bufs=1, space="DRAM") as dpool,
        ):
            # ---- stage the small inputs (chain-critical DMAs first) -----
            vecs_sb = cpool.tile([128, 68], f32)
            nc.sync.dma_start(vecs_sb[:], vecs.ap())
            aux_sb = cpool.tile([1, 128], f32)
            nc.sync.dma_start(aux_sb[:], aux_row.ap())
            wwt_sb = cpool.tile([128, 16, 64], f32)
            nc.sync.dma_start(
                wwt_sb[:], wwt.ap().rearrange("(c p) f -> p c f", p=128))
            enc_sb = cpool.tile([64, 1024], f32)
            nc.sync.dma_start(enc_sb[:], enc.ap())
            wct_sb = cpool.tile([128, 16, 128], f32)
            nc.sync.dma_start(
                wct_sb[:], wct.ap().rearrange("(c p) f -> p c f", p=128))
            wiht_sb = cpool.tile([128, 3072], f32)
            nc.sync.dma_start(wiht_sb[:], wiht.ap())
            whht_sb = cpool.tile([128, 3072], f32)
            nc.sync.dma_start(whht_sb[:], whht.ap())
            hloc_sb = cpool.tile([128, 1], f32)
            nc.sync.dma_start(hloc_sb[:], hloc.ap())
            bo_sb = cpool.tile([1, V_SH], f32)
            nc.sync.dma_start(bo_sb[:], bo_sh.ap())

            # ---- big Wo^T shard: 8 contraction tiles, streamed early ----
            wot_sb = cpool.tile([128, 8, V_SH], bf16)
            for k in range(8):
                nc.sync.dma_start(
                    wot_sb[:, k, :], wot.ap()[k * 128:(k + 1) * 128, :])

            ones_col = vecs_sb[:, 32:33]

            # ---- attention scores s = Ww_sh @ cat1 + bw_sh  ([64,1]) ----
            s_ps = sp_pool.tile([64, 1], f32, tag="sp")
            for c in range(16):
                nc.tensor.matmul(s_ps[:], wwt_sb[:, c, :],
                                 vecs_sb[:, c:c + 1],
                                 start=(c == 0), stop=(c == 15))
            s_sb = wpool.tile([64, 1], f32, tag="s_sb")
            nc.scalar.activation(s_sb[:], s_ps[:], AF.Identity,
                                 bias=vecs_sb[0:64, 65:66])
            exp_loc = wpool.tile([64, 1], f32, tag="exp_loc")
            nc.scalar.activation(exp_loc[:], s_sb[:], AF.Exp)
            # local sum of exp over the 64 partitions (matmul with ones)
            S_ps = sp_pool.tile([1, 1], f32, tag="sp")
            nc.tensor.matmul(S_ps[:], ones_col[0:64, :], exp_loc[:],
                             start=True, stop=True)
            spack = wpool.tile([1, 8], f32, tag="spack")
            nc.vector.memset(spack[:], 0.0)
            nc.scalar.copy(spack[0:1, 0:1], S_ps[:])

            # ---- AllGather #1: [s_local(64) | S_local | pad] -> [8,72] --
            cc1_in = dpool.tile([1, 72], f32)
            cc1_out = dpool.tile([8, 72], f32)
            nc.gpsimd.dma_start(cc1_in[0:1, 0:64], s_sb[:])
            nc.gpsimd.dma_start(cc1_in[0:1, 64:72], spack[:])
            nc.gpsimd.collective_compute(
                "AllGather", mybir.AluOpType.bypass, replica_groups=RG,
                ins=[cc1_in.opt()], outs=[cc1_out.opt()])

            # global softmax denominator
            srow8 = wpool.tile([1, 8], f32, tag="srow8")
            nc.gpsimd.dma_start(srow8[:], cc1_out.opt()[:, 64:65])
            S_tot = wpool.tile([1, 1], f32, tag="S_tot")
            nc.vector.reduce_sum(S_tot[:], srow8[:], axis=mybir.AxisListType.X)
            rinv = wpool.tile([1, 1], f32, tag="rinv")
            nc.vector.reciprocal(rinv[:], S_tot[:])
            rinv64_ps = sp_pool.tile([64, 1], f32, tag="sp")
            nc.tensor.matmul(rinv64_ps[:], aux_sb[0:1, 0:64], rinv[:],
                             start=True, stop=True)
            rinv64 = wpool.tile([64, 1], f32, tag="rinv64")
            nc.scalar.copy(rinv64[:], rinv64_ps[:])
            w_loc = wpool.tile([64, 1], f32, tag="w_loc")
            nc.vector.tensor_scalar_mul(w_loc[:], exp_loc[:], rinv64[:])

            # ---- weighted context (partial over local L rows) -----------
            wctx_ps = sp_pool.tile([128, 8], f32, tag="sp")
            for m in range(8):
                nc.tensor.matmul(wctx_ps[:, m:m + 1],
                                 enc_sb[:, m * 128:(m + 1) * 128], w_loc[:],
                                 start=True, stop=True)
            wctx_sb = wpool.tile([128, 8], f32, tag="wctx_sb")
            nc.scalar.copy(wctx_sb[:], wctx_ps[:])

            # ---- AllReduce #1: wctx ------------------------------------
            ccw_in = dpool.tile([128, 8], f32)
            ccw_out = dpool.tile([128, 8], f32)
            nc.gpsimd.dma_start(ccw_in[:], wctx_sb[:])
            nc.gpsimd.collective_compute(
                "AllReduce", mybir.AluOpType.add, replica_groups=RG,
                ins=[ccw_in.opt()], outs=[ccw_out.opt()])
            wctx_col = wpool.tile([128, 8], f32, tag="wctx_col")
            nc.gpsimd.dma_start(wctx_col[:], ccw_out.opt())

            # ---- out = relu(Wc_sh @ [embed; wctx] + bc_sh)  ([128,1]) ---
            o_ps = sp_pool.tile([128, 1], f32, tag="sp")
            for c in range(16):
                rhs = vecs_sb[:, 16 + c:17 + c] if c < 8 else \
                    wctx_col[:, c - 8:c - 7]
                nc.tensor.matmul(o_ps[:], wct_sb[:, c, :], rhs,
                                 start=(c == 0), stop=(c == 15))
            relu_bf = wpool.tile([128, 1], f32, tag="relu_bf")
            nc.scalar.activation(relu_bf[:], o_ps[:], AF.Relu,
                                 bias=vecs_sb[:, 66:67])

            # ---- GRU gate partials (contraction over local H slice) -----
            gig_ps = sp_pool.tile([128, 24], f32, tag="sp")
            for m in range(24):
                nc.tensor.matmul(gig_ps[:, m:m + 1],
                                 wiht_sb[:, m * 128:(m + 1) * 128],
                                 relu_bf[:], start=True, stop=True)
            ghh_ps = sp_pool.tile([128, 24], f32, tag="sp")
            for m in range(24):
                nc.tensor.matmul(ghh_ps[:, m:m + 1],
                                 whht_sb[:, m * 128:(m + 1) * 128],
                                 hloc_sb[:], start=True, stop=True)
            gigh_sb = wpool.tile([128, 48], f32, tag="gigh_sb")
            nc.scalar.copy(gigh_sb[:, 0:24], gig_ps[:])
            nc.vector.tensor_copy(gigh_sb[:, 24:48], ghh_ps[:])

            # ---- AllReduce #2: [gi | gh] partials ----------------------
            ccg_in = dpool.tile([128, 48], f32)
            ccg_out = dpool.tile([128, 48], f32)
            nc.gpsimd.dma_start(ccg_in[:], gigh_sb[:])
            nc.gpsimd.collective_compute(
                "AllReduce", mybir.AluOpType.add, replica_groups=RG,
                ins=[ccg_in.opt()], outs=[ccg_out.opt()])
            gigh = wpool.tile([128, 48], f32, tag="gigh")
            nc.gpsimd.dma_start(gigh[:], ccg_out.opt())

            # ---- gates: r,z = sig(gi+gh+b); n = tanh(gi_n+b + r*(gh_n+b))
            trz = wpool.tile([128, 16], f32, tag="trz")
            nc.vector.tensor_add(trz[:], gigh[:, 0:16], gigh[:, 24:40])
            trz2 = wpool.tile([128, 16], f32, tag="trz2")
            nc.vector.tensor_add(trz2[:], trz[:], vecs_sb[:, 33:49])
            rz = wpool.tile([128, 16], f32, tag="rz")
            nc.scalar.activation(rz[:], trz2[:], AF.Sigmoid)
            ghn = wpool.tile([128, 8], f32, tag="ghn")
            nc.vector.tensor_add(ghn[:], gigh[:, 40:48], vecs_sb[:, 57:65])
            tn = wpool.tile([128, 8], f32, tag="tn")
            nc.vector.tensor_mul(tn[:], rz[:, 0:8], ghn[:])
            tn2 = wpool.tile([128, 8], f32, tag="tn2")
            nc.vector.tensor_add(tn2[:], tn[:], gigh[:, 16:24])
            tn3 = wpool.tile([128, 8], f32, tag="tn3")
            nc.vector.tensor_add(tn3[:], tn2[:], vecs_sb[:, 49:57])
            nn_t = wpool.tile([128, 8], f32, tag="nn_t")
            nc.scalar.activation(nn_t[:], tn3[:], AF.Tanh)
            dd = wpool.tile([128, 8], f32, tag="dd")
            nc.vector.tensor_sub(dd[:], vecs_sb[:, 24:32], nn_t[:])
            ee = wpool.tile([128, 8], f32, tag="ee")
            nc.vector.tensor_mul(ee[:], rz[:, 8:16], dd[:])
            hnew = wpool.tile([128, 8], f32, tag="hnew")
            nc.vector.tensor_add(hnew[:], nn_t[:], ee[:])
            nc.gpsimd.dma_start(hnew_o.ap(), hnew[:])
            hnew_bf = wpool.tile([128, 8], bf16, tag="hnew_bf")
            nc.vector.tensor_copy(hnew_bf[:], hnew[:])

            # ---- logits slice: 13 chunks of <=512, accumulate 8 k-tiles -
            logits_row = cpool.tile([1, V_SH], f32)
            sumexps = cpool.tile([1, 16], f32)
            for c in range(N_CH):
                c0 = c * CH
                csz = min(CH, V_SH - c0)
                lg_ps = lg_pool.tile([1, CH], f32, tag="lg")
                for k in range(8):
                    nc.tensor.matmul(lg_ps[0:1, 0:csz],
                                     hnew_bf[:, k:k + 1],
                                     wot_sb[:, k, c0:c0 + csz],
                                     start=(k == 0), stop=(k == 7))
                # logits = psum + bo  (into the row buffer), then exp+accum
                nc.vector.tensor_add(logits_row[0:1, c0:c0 + csz],
                                     lg_ps[0:1, 0:csz],
                                     bo_sb[0:1, c0:c0 + csz])
                etmp = wpool.tile([1, CH], f32, tag="etmp")
                nc.scalar.activation(etmp[0:1, 0:csz],
                                     logits_row[0:1, c0:c0 + csz], AF.Exp,
                                     accum_out=sumexps[0:1, c:c + 1])

            sumloc = wpool.tile([1, 1], f32, tag="sumloc")
            nc.vector.reduce_sum(sumloc[:], sumexps[0:1, 0:N_CH],
                                 axis=mybir.AxisListType.X)
            spack2 = wpool.tile([1, 8], f32, tag="spack2")
            nc.vector.memset(spack2[:], 0.0)
            nc.vector.tensor_copy(spack2[0:1, 0:1], sumloc[:])

            # ---- AllGather #2: logit sumexp ----------------------------
            ccs_in = dpool.tile([1, 8], f32)
            ccs_out = dpool.tile([8, 8], f32)
            nc.gpsimd.dma_start(ccs_in[:], spack2[:])
            nc.gpsimd.collective_compute(
                "AllGather", mybir.AluOpType.bypass, replica_groups=RG,
                ins=[ccs_in.opt()], outs=[ccs_out.opt()])
            s2row = wpool.tile([1, 8], f32, tag="s2row")
            nc.gpsimd.dma_start(s2row[:], ccs_out.opt()[:, 0:1])
            S2 = wpool.tile([1, 1], f32, tag="S2")
            nc.vector.reduce_sum(S2[:], s2row[:], axis=mybir.AxisListType.X)
            logS = wpool.tile([1, 1], f32, tag="logS")
            nc.scalar.activation(logS[:], S2[:], AF.Ln)
            nlogS = wpool.tile([1, 1], f32, tag="nlogS")
            nc.scalar.activation(nlogS[:], logS[:], AF.Copy, scale=-1.0)

            # out = logits - log(sum); split across DVE and ACT
            nc.vector.tensor_scalar_sub(logits_row[0:1, 0:SUB_SPLIT],
                                        logits_row[0:1, 0:SUB_SPLIT],
                                        logS[:])
            nc.scalar.activation(logits_row[0:1, SUB_SPLIT:V_SH],
                                 logits_row[0:1, SUB_SPLIT:V_SH],
                                 AF.Identity, bias=nlogS[:])
            nc.sync.dma_start(out_sl.ap(), logits_row[:])

            # ---- attention weights output (off the critical path) -------
            srow_all = wpool.tile([1, 576], f32, tag="srow_all")
            nc.gpsimd.dma_start(srow_all[:], cc1_out.opt())
            eall = wpool.tile([1, 512], f32, tag="eall")
            nc.scalar.activation(
                eall[0:1, :].rearrange("p (r m) -> p r m", m=64),
                srow_all[0:1, :].rearrange("p (r m) -> p r m", m=72)[:, :, 0:64],
                AF.Exp)
            wrow = wpool.tile([1, 512], f32, tag="wrow")
            nc.vector.tensor_scalar_mul(wrow[:], eall[:], rinv[:])
            nc.gpsimd.dma_start(w_o.ap(), wrow[:])

    nc.compile()
    return nc


def _col(v, ncols):
    return np.ascontiguousarray(v.reshape(ncols, 128).T)


def _prep_in_maps(inputs):
    f32 = np.float32
    x = np.asarray(inputs["x"]).reshape(-1)
    hidden = np.asarray(inputs["hidden"], f32).reshape(H)
    enc_full = np.asarray(inputs["encoder_outputs"], f32)
    emb = np.asarray(inputs["emb"], f32)
    Ww = np.asarray(inputs["Ww"], f32)
    bw = np.asarray(inputs["bw"], f32)
    Wc = np.asarray(inputs["Wc"], f32)
    bc = np.asarray(inputs["bc"], f32)
    Wih = np.asarray(inputs["Wih"], f32)
    Whh = np.asarray(inputs["Whh"], f32)
    bih = np.asarray(inputs["bih"], f32)
    bhh = np.asarray(inputs["bhh"], f32)
    Wo = np.asarray(inputs["Wo"], f32)
    bo = np.asarray(inputs["bo"], f32)

    embed = emb[int(x[0])]
    cat1 = np.concatenate([embed, hidden])

    vecs = np.zeros((128, 68), f32)
    vecs[:, 0:16] = _col(cat1, 16)
    vecs[:, 16:24] = _col(embed, 8)
    vecs[:, 24:32] = _col(hidden, 8)
    vecs[:, 32] = 1.0
    vecs[:, 33:49] = _col((bih + bhh)[0:2048], 16)
    vecs[:, 49:57] = _col(bih[2048:], 8)
    vecs[:, 57:65] = _col(bhh[2048:], 8)

    aux = np.ones((1, 128), f32)

    pad = V_PAD - V
    Wo_pad = np.concatenate([Wo, np.zeros((pad, H), f32)], axis=0)
    bo_pad = np.concatenate([bo, np.full((pad,), -1e4, f32)])

    in_maps = []
    for r in range(NCORES):
        vr = vecs.copy()
        vr[0:64, 65] = bw[r * 64:(r + 1) * 64]
        vr[:, 66] = bc[r * 128:(r + 1) * 128]
        hs = slice(r * 128, (r + 1) * 128)
        vs = slice(r * V_SH, (r + 1) * V_SH)
        in_maps.append({
            "vecs": vr,
            "aux_row": aux,
            "wwt": np.ascontiguousarray(Ww[r * 64:(r + 1) * 64, :].T),
            "enc": np.ascontiguousarray(enc_full[r * 64:(r + 1) * 64, :]),
            "wct": np.ascontiguousarray(Wc[hs, :].T),
            "wiht": np.ascontiguousarray(Wih[:, hs].T),
            "whht": np.ascontiguousarray(Whh[:, hs].T),
            "hloc": hidden[hs].reshape(128, 1).copy(),
            "wot": np.ascontiguousarray(Wo_pad[vs, :].T).astype(_BF16),
            "bo_sh": bo_pad[vs].reshape(1, V_SH),
        })
    return in_maps


def _get_nc():
    if "nc" not in _cache:
        _cache["nc"] = _build()
    return _cache["nc"]


def _assemble(results):
    out = np.concatenate(
        [results[r]["out_sl"].reshape(-1) for r in range(NCORES)])[:V]
    out = np.ascontiguousarray(out.reshape(1, V), dtype=np.float32)
    h_new = np.ascontiguousarray(
        results[0]["hnew_o"].T.reshape(1, 1, H), dtype=np.float32)
    weights = np.ascontiguousarray(
        results[0]["w_o"].reshape(1, L), dtype=np.float32)
    return out, h_new, weights


def kernel(**inputs):
    from concourse.bass_utils import run_bass_kernel_spmd

    nc = _get_nc()
    in_maps = _prep_in_maps(inputs)
    res = run_bass_kernel_spmd(nc, in_maps, list(range(NCORES)))
    return _assemble(res.results)


# revision 5
# speedup vs baseline: 1.4395x; 1.0198x over previous
"""Trainium2 Bass kernel for nn_DecoderGRUWeighted (batch-1 GRU decoder step).

Strategy (8 NeuronCores, SPMD):
  - Vocab dim of the output projection Wo (50257x1024, the dominant memory
    traffic) is sharded 8 ways; each core computes a [1, V/8] logits slice
    with a 4-way column-tiled PE matmul (4 concurrent streams).
  - Attention scores are sharded by L and combined with one tiny AllGather;
    each core then computes the full softmax + context locally (encoder
    replicated). GRU gate partials are combined with one AllReduce.
  - A dependency-free dummy AllGather fires at t=0 so the cross-core
    entry barrier + first-collective setup overlap the weight streaming.
  - All length-D vectors live on chip in "column layout": SBUF tile
    [128, D/128] with element d at (partition d%128, column d//128), so every
    matvec uses natural [128,128] weight tiles as the PE stationary operand
    and [128,1] vector columns as the moving operand, with no transposes.
  - log_softmax is computed without max subtraction (logits are O(1) here):
    out = logits - ln(sum_exp), with the global sum reduced via AllGather.
  - Wo^T is stored bf16 on chip to halve HBM traffic; all the small
    weights stay f32.
"""

import sys

if "/opt/trn_rl_repo" not in sys.path:
    sys.path.insert(0, "/opt/trn_rl_repo")

import numpy as np
import ml_dtypes

H = 1024
V = 50257
L = 512
NCORES = 8
V_SH = 6283          # ceil(V / 8); global pad = 50264 (7 zero rows on core 7)
V_PAD = V_SH * NCORES
CH = 512             # logits chunk (one PSUM bank of f32)
N_CH = 13            # 12*512 + 139
G_COLS = 1675        # per-group row length: 3*512 + 139 (group 0 holds the tail)

_BF16 = ml_dtypes.bfloat16

_cache: dict = {}


def _build():
    import concourse.bacc as bacc
    import concourse.tile as tile
    from concourse import mybir

    f32 = mybir.dt.float32
    bf16 = mybir.dt.bfloat16
    AF = mybir.ActivationFunctionType

    nc = bacc.Bacc("TRN2", target_bir_lowering=False, debug=False,
                   num_devices=NCORES)

    # ---- I/O ------------------------------------------------------------
    # vecs columns: 0:16 cat1 | 16:24 embed | 24:32 hidden | 32 ones |
    #               33:49 bih+bhh (r,z) | 49:57 bih_n | 57:65 bhh_n |
    #               65 bw shard (partitions 0:64) | 66 bc shard
    vecs = nc.dram_tensor("vecs", [128, 68], f32, kind="ExternalInput")
    aux_row = nc.dram_tensor("aux_row", [1, 128], f32, kind="ExternalInput")
    wwt = nc.dram_tensor("wwt", [2048, 64], f32, kind="ExternalInput")
    enc = nc.dram_tensor("enc", [512, 1024], f32, kind="ExternalInput")
    wct = nc.dram_tensor("wct", [2048, 128], f32, kind="ExternalInput")
    wiht = nc.dram_tensor("wiht", [128, 3072], f32, kind="ExternalInput")
    whht = nc.dram_tensor("whht", [128, 3072], f32, kind="ExternalInput")
    hloc = nc.dram_tensor("hloc", [128, 1], f32, kind="ExternalInput")
    wot = nc.dram_tensor("wot", [1024, V_SH], bf16, kind="ExternalInput")
    bo4 = nc.dram_tensor("bo4", [4, G_COLS], f32, kind="ExternalInput")

    out_sl = nc.dram_tensor("out_sl", [1, V_SH], f32, kind="ExternalOutput")
    hnew_o = nc.dram_tensor("hnew_o", [128, 8], f32, kind="ExternalOutput")
    w_o = nc.dram_tensor("w_o", [1, 512], f32, kind="ExternalOutput")

    RG = [list(range(NCORES))]

    with tile.TileContext(nc) as tc:
        with (
            tc.tile_pool(name="const", bufs=1) as cpool,
            tc.tile_pool(name="work", bufs=2) as wpool,
            tc.tile_pool(name="lgp", bufs=4, space="PSUM") as lg_pool,
            tc.tile_pool(name="spp", bufs=2, space="PSUM") as sp_pool,
            tc.tile_pool(name="dram", bufs=1, space="DRAM") as dpool,
        ):
            # ---- dummy collective at t=0: absorbs the entry barrier -----
            zz = cpool.tile([1, 8], f32)
            nc.vector.memset(zz[:], 0.0)
            dummy_in = dpool.tile([1, 8], f32)
            dummy_out = dpool.tile([8, 8], f32)
            nc.gpsimd.dma_start(dummy_in[:], zz[:])
            nc.gpsimd.collective_compute(
                "AllGather", mybir.AluOpType.bypass, replica_groups=RG,
                ins=[dummy_in.opt()], outs=[dummy_out.opt()])

            # ---- stage the small inputs (chain-critical DMAs first) -----
            vecs_sb = cpool.tile([128, 68], f32)
            nc.sync.dma_start(vecs_sb[:], vecs.ap())
            aux_sb = cpool.tile([1, 128], f32)
            nc.sync.dma_start(aux_sb[:], aux_row.ap())
            wwt_sb = cpool.tile([128, 16, 64], f32)
            nc.sync.dma_start(
                wwt_sb[:], wwt.ap().rearrange("(c p) f -> p c f", p=128))
            enc_sb = cpool.tile([128, 4, 1024], f32)
            nc.sync.dma_start(
                enc_sb[:], enc.ap().rearrange("(c p) f -> p c f", p=128))
            wct_sb = cpool.tile([128, 16, 128], f32)
            nc.sync.dma_start(
                wct_sb[:], wct.ap().rearrange("(c p) f -> p c f", p=128))
            wiht_sb = cpool.tile([128, 3072], f32)
            nc.sync.dma_start(wiht_sb[:], wiht.ap())
            whht_sb = cpool.tile([128, 3072], f32)
            nc.sync.dma_start(whht_sb[:], whht.ap())
            hloc_sb = cpool.tile([128, 1], f32)
            nc.sync.dma_start(hloc_sb[:], hloc.ap())
            bo4_sb = cpool.tile([128, G_COLS], f32)
            nc.sync.dma_start(
                bo4_sb[:, :].rearrange("(a b) f -> a b f", b=32)[:, 0:1, :],
                bo4.ap())

            # ---- big Wo^T shard: 8 contraction tiles, streamed early ----
            wot_sb = cpool.tile([128, 8, V_SH], bf16)
            for k in range(8):
                nc.sync.dma_start(
                    wot_sb[:, k, :], wot.ap()[k * 128:(k + 1) * 128, :])

            ones_col = vecs_sb[:, 32:33]

            # ---- attention scores s = Ww_sh @ cat1 + bw_sh  ([64,1]) ----
            s_ps = sp_pool.tile([64, 1], f32, tag="sp")
            for c in range(16):
                nc.tensor.matmul(s_ps[:], wwt_sb[:, c, :],
                                 vecs_sb[:, c:c + 1],
                                 start=(c == 0), stop=(c == 15))
            s_sb = wpool.tile([64, 1], f32, tag="s_sb")
            nc.scalar.activation(s_sb[:], s_ps[:], AF.Identity,
                                 bias=vecs_sb[0:64, 65:66])

            # ---- AllGather #1: local scores -> all 512 scores -----------
            cc1_in = dpool.tile([1, 64], f32)
            cc1_out = dpool.tile([8, 64], f32)
            nc.gpsimd.dma_start(cc1_in[:], s_sb[:])
            nc.gpsimd.collective_compute(
                "AllGather", mybir.AluOpType.bypass, replica_groups=RG,
                ins=[cc1_in.opt()], outs=[cc1_out.opt()])

            # readback into column layout [128, 4]: element l=128t+64e+j at
            # (partition 64e+j, col t); gathered row r=2t+e holds j=0..63
            scores_col = wpool.tile([128, 4], f32, tag="scores_col")
            cc1_v = cc1_out.opt().rearrange("(t e) j -> e j t", e=2)
            nc.gpsimd.dma_start(scores_col[0:64, 0:4], cc1_v[0:1])
            nc.gpsimd.dma_start(scores_col[64:128, 0:4], cc1_v[1:2])

            # full softmax (local): w = exp(s) / sum(exp(s))
            exp4 = wpool.tile([128, 4], f32, tag="exp4")
            acc4 = wpool.tile([128, 1], f32, tag="acc4")
            nc.scalar.activation(exp4[:], scores_col[:], AF.Exp,
                                 accum_out=acc4[:])
            S_ps = sp_pool.tile([1, 1], f32, tag="sp")
            nc.tensor.matmul(S_ps[:], ones_col[:], acc4[:],
                             start=True, stop=True)
            S_att = wpool.tile([1, 1], f32, tag="S_att")
            nc.scalar.copy(S_att[:], S_ps[:])
            rinv = wpool.tile([1, 1], f32, tag="rinv")
            nc.vector.reciprocal(rinv[:], S_att[:])
            rb_ps = sp_pool.tile([128, 1], f32, tag="sp")
            nc.tensor.matmul(rb_ps[:], aux_sb[0:1, 0:128], rinv[:],
                             start=True, stop=True)
            rinv128 = wpool.tile([128, 1], f32, tag="rinv128")
            nc.scalar.copy(rinv128[:], rb_ps[:])
            w_col = wpool.tile([128, 4], f32, tag="w_col")
            nc.vector.tensor_scalar_mul(w_col[:], exp4[:], rinv128[:])

            # ---- full weighted context wctx = w @ enc  ([128,8] col) ----
            wctx_ps = sp_pool.tile([128, 8], f32, tag="sp")
            for m in range(8):
                for lc in range(4):
                    nc.tensor.matmul(wctx_ps[:, m:m + 1],
                                     enc_sb[:, lc, m * 128:(m + 1) * 128],
                                     w_col[:, lc:lc + 1],
                                     start=(lc == 0), stop=(lc == 3))
            wctx_col = wpool.tile([128, 8], f32, tag="wctx_col")
            nc.scalar.copy(wctx_col[:], wctx_ps[:])

            # ---- out = relu(Wc_sh @ [embed; wctx] + bc_sh)  ([128,1]) ---
            o_ps = sp_pool.tile([128, 1], f32, tag="sp")
            for c in range(16):
                rhs = vecs_sb[:, 16 + c:17 + c] if c < 8 else \
                    wctx_col[:, c - 8:c - 7]
                nc.tensor.matmul(o_ps[:], wct_sb[:, c, :], rhs,
                                 start=(c == 0), stop=(c == 15))
            relu_sb = wpool.tile([128, 1], f32, tag="relu_sb")
            nc.scalar.activation(relu_sb[:], o_ps[:], AF.Relu,
                                 bias=vecs_sb[:, 66:67])

            # ---- GRU gate partials (contraction over local H slice) -----
            gig_ps = sp_pool.tile([128, 24], f32, tag="sp")
            for m in range(24):
                nc.tensor.matmul(gig_ps[:, m:m + 1],
                                 wiht_sb[:, m * 128:(m + 1) * 128],
                                 relu_sb[:], start=True, stop=True)
            ghh_ps = sp_pool.tile([128, 24], f32, tag="sp")
            for m in range(24):
                nc.tensor.matmul(ghh_ps[:, m:m + 1],
                                 whht_sb[:, m * 128:(m + 1) * 128],
                                 hloc_sb[:], start=True, stop=True)
            gigh_sb = wpool.tile([128, 48], f32, tag="gigh_sb")
            nc.scalar.copy(gigh_sb[:, 0:24], gig_ps[:])
            nc.vector.tensor_copy(gigh_sb[:, 24:48], ghh_ps[:])

            # ---- AllReduce: [gi | gh] partials --------------------------
            ccg_in = dpool.tile([128, 48], f32)
            ccg_out = dpool.tile([128, 48], f32)
            nc.gpsimd.dma_start(ccg_in[:], gigh_sb[:])
            nc.gpsimd.collective_compute(
                "AllReduce", mybir.AluOpType.add, replica_groups=RG,
                ins=[ccg_in.opt()], outs=[ccg_out.opt()])
            gigh = wpool.tile([128, 48], f32, tag="gigh")
            nc.gpsimd.dma_start(gigh[:], ccg_out.opt())

            # ---- gates: r,z = sig(gi+gh+b); n = tanh(gi_n+b + r*(gh_n+b))
            trz = wpool.tile([128, 16], f32, tag="trz")
            nc.vector.tensor_add(trz[:], gigh[:, 0:16], gigh[:, 24:40])
            trz2 = wpool.tile([128, 16], f32, tag="trz2")
            nc.vector.tensor_add(trz2[:], trz[:], vecs_sb[:, 33:49])
            rz = wpool.tile([128, 16], f32, tag="rz")
            nc.scalar.activation(rz[:], trz2[:], AF.Sigmoid)
            ghn = wpool.tile([128, 8], f32, tag="ghn")
            nc.vector.tensor_add(ghn[:], gigh[:, 40:48], vecs_sb[:, 57:65])
            tn = wpool.tile([128, 8], f32, tag="tn")
            nc.vector.tensor_mul(tn[:], rz[:, 0:8], ghn[:])
            tn2 = wpool.tile([128, 8], f32, tag="tn2")
            nc.vector.tensor_add(tn2[:], tn[:], gigh[:, 16:24])
            tn3 = wpool.tile([128, 8], f32, tag="tn3")
            nc.vector.tensor_add(tn3[:], tn2[:], vecs_sb[:, 49:57])
            nn_t = wpool.tile([128, 8], f32, tag="nn_t")
            nc.scalar.activation(nn_t[:], tn3[:], AF.Tanh)
            dd = wpool.tile([128, 8], f32, tag="dd")
            nc.vector.tensor_sub(dd[:], vecs_sb[:, 24:32], nn_t[:])
            ee = wpool.tile([128, 8], f32, tag="ee")
            nc.vector.tensor_mul(ee[:], rz[:, 8:16], dd[:])
            hnew = wpool.tile([128, 8], f32, tag="hnew")
            nc.vector.tensor_add(hnew[:], nn_t[:], ee[:])
            nc.gpsimd.dma_start(hnew_o.ap(), hnew[:])
            hnew_bf = wpool.tile([128, 8], bf16, tag="hnew_bf")
            nc.vector.tensor_copy(hnew_bf[:], hnew[:])

            # ---- logits: 4-way column-tiled matmul ----------------------
            # chunk c = 4*cc + g -> PE column-group g, output partition 32g,
            # logits4 row 32g columns [cc*512, cc*512+csz)
            logits4 = cpool.tile([128, G_COLS], f32)
            sumexp4 = cpool.tile([128, 4], f32)
            for cc in range(4):
                lg_ps = lg_pool.tile([128, CH], f32, tag="lg")
                for k in range(8):
                    for g in range(4):
                        c = 4 * cc + g
                        if c >= N_CH:
                            continue
                        csz = min(CH, V_SH - c * CH)
                        nc.tensor.matmul(lg_ps[32 * g:32 * g + 1, 0:csz],
                                         hnew_bf[:, k:k + 1],
                                         wot_sb[:, k,
                                                c * CH:c * CH + csz],
                                         start=(k == 0), stop=(k == 7),
                                         tile_position=(0, 32 * g))
                for g in range(4):
                    c = 4 * cc + g
                    if c >= N_CH:
                        continue
                    csz = min(CH, V_SH - c * CH)
                    p = 32 * g
                    row = logits4[p:p + 1, cc * CH:cc * CH + csz]
                    nc.vector.tensor_add(row, lg_ps[p:p + 1, 0:csz],
                                         bo4_sb[p:p + 1,
                                                cc * CH:cc * CH + csz])
                    etmp = wpool.tile([128, CH], f32, tag="etmp")
                    nc.scalar.activation(etmp[p:p + 1, 0:csz], row, AF.Exp,
                                         accum_out=sumexp4[p:p + 1,
                                                           cc:cc + 1])

            # ---- global sumexp via AllGather ----------------------------
            se4 = wpool.tile([128, 1], f32, tag="se4")
            for g in range(4):
                p = 32 * g
                ncc = 4 if g == 0 else 3
                nc.vector.reduce_sum(se4[p:p + 1, :],
                                     sumexp4[p:p + 1, 0:ncc],
                                     axis=mybir.AxisListType.X)
            ccs_in = dpool.tile([1, 8], f32)
            ccs_out = dpool.tile([8, 8], f32)
            nc.gpsimd.dma_start(ccs_in[:], zz[:])
            nc.gpsimd.dma_start(
                ccs_in[0:1, 0:4],
                se4[:, :].rearrange("(a b) f -> a b f", b=32)[:, 0:1, :])
            nc.gpsimd.collective_compute(
                "AllGather", mybir.AluOpType.bypass, replica_groups=RG,
                ins=[ccs_in.opt()], outs=[ccs_out.opt()])
            s2row = wpool.tile([1, 32], f32, tag="s2row")
            nc.gpsimd.dma_start(s2row[:], ccs_out.opt()[:, 0:4])
            S2 = wpool.tile([1, 1], f32, tag="S2")
            nc.vector.reduce_sum(S2[:], s2row[:], axis=mybir.AxisListType.X)
            logS = wpool.tile([1, 1], f32, tag="logS")
            nc.scalar.activation(logS[:], S2[:], AF.Ln)
            logS_row = wpool.tile([1, 4], f32, tag="logS_row")
            nc.vector.tensor_scalar_mul(logS_row[:], aux_sb[0:1, 0:4],
                                        logS[:])
            logs4 = wpool.tile([128, 1], f32, tag="logs4")
            nc.gpsimd.dma_start(
                logs4[:, :].rearrange("(a b) f -> a b f", b=32)[:, 0:1, :],
                logS_row[:])

            # out = logits - ln(S); per group row (DVE + ACT split)
            nlogS_row = wpool.tile([1, 4], f32, tag="nlogS_row")
            nc.scalar.activation(nlogS_row[:], logS_row[:], AF.Copy,
                                 scale=-1.0)
            nlogs4 = wpool.tile([128, 1], f32, tag="nlogs4")
            nc.gpsimd.dma_start(
                nlogs4[:, :].rearrange("(a b) f -> a b f", b=32)[:, 0:1, :],
                nlogS_row[:])
            for g in range(4):
                p = 32 * g
                csz = G_COLS if g == 0 else 3 * CH
                row = logits4[p:p + 1, 0:csz]
                if g % 2 == 0:
                    nc.vector.tensor_scalar_sub(row, row, logs4[p:p + 1, :])
                else:
                    nc.scalar.activation(row, row, AF.Identity,
                                         bias=nlogs4[p:p + 1, :])

            # ---- output DMAs -------------------------------------------
            src_main = logits4[:, 0:3 * CH].rearrange(
                "(a b) (cc f) -> a b cc f", b=32, f=CH)[:, 0:1, :, :]
            dst_main = out_sl.ap()[:, 0:4 * 3 * CH].rearrange(
                "p (cc g f) -> p g cc f", cc=3, g=4)
            nc.sync.dma_start(dst_main, src_main)
            nc.sync.dma_start(out_sl.ap()[:, 12 * CH:V_SH],
                              logits4[0:1, 3 * CH:G_COLS])

            # ---- attention weights output (off the critical path) -------
            # w_o element d=128t+64e+j <- w_col[64e+j, t]
            wo_v = w_o.ap().rearrange("p (t e j) -> p e j t", e=2, j=64)
            nc.gpsimd.dma_start(wo_v[:, 0:1], w_col[0:64, 0:4])
            nc.gpsimd.dma_start(wo_v[:, 1:2], w_col[64:128, 0:4])

    nc.compile()
    return nc


def _col(v, ncols):
    return np.ascontiguousarray(v.reshape(ncols, 128).T)


def _prep_in_maps(inputs):
    f32 = np.float32
    x = np.asarray(inputs["x"]).reshape(-1)
    hidden = np.asarray(inputs["hidden"], f32).reshape(H)
    enc_full = np.ascontiguousarray(np.asarray(inputs["encoder_outputs"], f32))
    emb = np.asarray(inputs["emb"], f32)
    Ww = np.asarray(inputs["Ww"], f32)
    bw = np.asarray(inputs["bw"], f32)
    Wc = np.asarray(inputs["Wc"], f32)
    bc = np.asarray(inputs["bc"], f32)
    Wih = np.asarray(inputs["Wih"], f32)
    Whh = np.asarray(inputs["Whh"], f32)
    bih = np.asarray(inputs["bih"], f32)
    bhh = np.asarray(inputs["bhh"], f32)
    Wo = np.asarray(inputs["Wo"], f32)
    bo = np.asarray(inputs["bo"], f32)

    embed = emb[int(x[0])]
    cat1 = np.concatenate([embed, hidden])

    vecs = np.zeros((128, 68), f32)
    vecs[:, 0:16] = _col(cat1, 16)
    vecs[:, 16:24] = _col(embed, 8)
    vecs[:, 24:32] = _col(hidden, 8)
    vecs[:, 32] = 1.0
    vecs[:, 33:49] = _col((bih + bhh)[0:2048], 16)
    vecs[:, 49:57] = _col(bih[2048:], 8)
    vecs[:, 57:65] = _col(bhh[2048:], 8)

    aux = np.ones((1, 128), f32)

    pad = V_PAD - V
    Wo_pad = np.concatenate([Wo, np.zeros((pad, H), f32)], axis=0)
    bo_pad = np.concatenate([bo, np.full((pad,), -1e4, f32)])

    in_maps = []
    for r in range(NCORES):
        vr = vecs.copy()
        vr[0:64, 65] = bw[r * 64:(r + 1) * 64]
        vr[:, 66] = bc[r * 128:(r + 1) * 128]
        hs = slice(r * 128, (r + 1) * 128)
        bo_sh = bo_pad[r * V_SH:(r + 1) * V_SH]
        bo4m = np.zeros((4, G_COLS), f32)
        for c in range(N_CH):
            g, cc = c % 4, c // 4
            csz = min(CH, V_SH - c * CH)
            bo4m[g, cc * CH:cc * CH + csz] = bo_sh[c * CH:c * CH + csz]
        in_maps.append({
            "vecs": vr,
            "aux_row": aux,
            "wwt": np.ascontiguousarray(Ww[r * 64:(r + 1) * 64, :].T),
            "enc": enc_full,
            "wct": np.ascontiguousarray(Wc[hs, :].T),
            "wiht": np.ascontiguousarray(Wih[:, hs].T),
            "whht": np.ascontiguousarray(Whh[:, hs].T),
            "hloc": hidden[hs].reshape(128, 1).copy(),
            "wot": np.ascontiguousarray(
                Wo_pad[r * V_SH:(r + 1) * V_SH, :].T).astype(_BF16),
            "bo4": bo4m,
        })
    return in_maps


def _get_nc():
    if "nc" not in _cache:
        _cache["nc"] = _build()
    return _cache["nc"]


def _assemble(results):
    out = np.concatenate(
        [results[r]["out_sl"].reshape(-1) for r in range(NCORES)])[:V]
    out = np.ascontiguousarray(out.reshape(1, V), dtype=np.float32)
    h_new = np.ascontiguousarray(
        results[0]["hnew_o"].T.reshape(1, 1, H), dtype=np.float32)
    weights = np.ascontiguousarray(
        results[0]["w_o"].reshape(1, L), dtype=np.float32)
    return out, h_new, weights


def kernel(**inputs):
    from concourse.bass_utils import run_bass_kernel_spmd

    nc = _get_nc()
    in_maps = _prep_in_maps(inputs)
    res = run_bass_kernel_spmd(nc, in_maps, list(range(NCORES)))
    return _assemble(res.results)
